# revision 21
# baseline (speedup 1.0000x reference)
"""ChatGLM2 attention block (B=2, S=2048, H=4096, 32 q heads / 2 kv heads,
head_dim=128, partial interleaved RoPE) on 8 Trainium2 NeuronCores.

Sharding: tensor-parallel over heads. Core c owns q heads 4c..4c+3 and the
kv head c//4 (cores 0-3 -> kv0, 4-7 -> kv1). Each core computes its QKV
shard, causal attention for its 4 heads, and a partial dense projection
(contraction over its 512 ctx dims). Host sums the 8 partial outputs.

All matmuls run in float32r (TF32) at full PE rate. Everything on-device is
kept transposed ([feature, token]) so the PE contraction dim is always the
partition dim; the host transposes once at the end.
"""
import sys
import types

import numpy as np

sys.path.insert(0, "/opt/trn_rl_repo")

# NTFF profile hook (the image's antenv lacks axon_hooks; bass_utils wants it
# when trace=True). Registering it is harmless when tracing is off.
try:  # pragma: no cover - only matters when profiling
    import trn_agent_boot.trn_boot as _tb

    _m = types.ModuleType("antenv.axon_hooks")
    _hook = _tb._ntff_profile_via_ctypes("/opt/axon/libaxon_pjrt.so")
    _m.get_axon_ntff_profile_hook = lambda: _hook
    _m.set_axon_ntff_profile_hook = lambda h: None
    sys.modules.setdefault("antenv.axon_hooks", _m)
except Exception:
    pass

import concourse.bass as bass
import concourse.tile as tile
from concourse import bacc, mybir
from concourse.bass_utils import run_bass_kernel_spmd

F32 = mybir.dt.float32
F32R = mybir.dt.float32r

N_CORES = 8
HEAD_DIM = 128
ROT = 64          # rotary dims per head (first half, interleaved pairs)
KV_HEADS = 2
ROPE_BASE = 10000.0

# full-problem sizes
FULL = dict(B=2, S=2048, H=4096, NQH=4)

TOKW = 256        # phase-1 token slice width
QW = 512          # attention q-tile width / matmul free dim


def build_core_kernel(S, H, NQH, debug_spill=False):
    """One core's program. S = seq len per batch, H = hidden, NQH = q heads
    per core. T = 2*S tokens. Returns a compiled Bacc."""
    T = 2 * S
    KT = H // 128               # qkv contraction tiles
    QKV_M = NQH + 2             # per-core qkv output row tiles (q heads, k, v)
    QR = NQH * 128              # q rows
    NS = T // TOKW              # phase-1 token slices
    NQT = S // QW               # q tiles per batch
    NVT = S // 128              # v/k token tiles per batch
    DKT = NQH                   # dense contraction tiles (per-core ctx dims / 128)

    nc = bacc.Bacc("TRN2", target_bir_lowering=False, debug=False)

    xT = nc.dram_tensor("xT", [H, T], F32R, kind="ExternalInput")
    wT = nc.dram_tensor("wT", [H, QKV_M * 128], F32R, kind="ExternalInput")
    bias = nc.dram_tensor("bias", [QKV_M * 128, 1], F32, kind="ExternalInput")
    dwT = nc.dram_tensor("dwT", [NQH * 128, H], F32R, kind="ExternalInput")
    cos4 = nc.dram_tensor("cos4", [128, T], F32R, kind="ExternalInput")
    sin4 = nc.dram_tensor("sin4", [128, T], F32R, kind="ExternalInput")
    maskt = nc.dram_tensor("maskt", [128, 4 * QW], F32R, kind="ExternalInput")
    ident = nc.dram_tensor("ident", [128, 128], F32R, kind="ExternalInput")
    onesc = nc.dram_tensor("onesc", [128, 128], F32R, kind="ExternalInput")
    outT = nc.dram_tensor("outT", [H, T], F32, kind="ExternalOutput")

    # per-batch spill tensors: a reader of batch b's tensor only has to wait
    # for batch b's phase-1/2 writes, so phases overlap across batches
    dbg_kind = dict(kind="ExternalOutput") if debug_spill else {}
    qkv_d = [nc.dram_tensor(f"qkvT{b}_d", [QKV_M * 128, S], F32R, **dbg_kind)
             for b in range(2)]
    ctx_d = [nc.dram_tensor(f"ctxT{b}_d", [NQH * 128, S], F32R, **dbg_kind)
             for b in range(2)]

    scale = 1.0 / float(np.sqrt(HEAD_DIM))
    Exp = mybir.ActivationFunctionType.Exp
    Ident = mybir.ActivationFunctionType.Identity

    with tile.TileContext(nc) as tc:
        # ---------------- phase 1: qkvT = W @ xT (+bias, +RoPE), spill ----
        with (
            tc.tile_pool(name="wp", bufs=1) as wp,
            tc.tile_pool(name="biasp", bufs=1) as biasp,
            tc.tile_pool(name="xp", bufs=2) as xp,
            tc.tile_pool(name="tabp", bufs=2) as tabp,
            tc.tile_pool(name="stp", bufs=2) as stp,
            tc.tile_pool(name="ropep", bufs=2) as ropep,
            tc.tile_pool(name="ps1", bufs=1, space="PSUM") as ps1,
        ):
            w_t = [wp.tile([128, QKV_M * 128], F32R, name=f"w{k}", tag=f"w{k}")
                   for k in range(KT)]
            b_t = [biasp.tile([128, 1], F32, name=f"b{m}", tag=f"b{m}")
                   for m in range(QKV_M)]

            xT_kp = xT.rearrange("(k p) t -> p k t", p=128)

            for n in range(NS):
                sl = slice(n * TOKW, (n + 1) * TOKW)
                bn = (n * TOKW) // S
                osl = slice(n * TOKW - bn * S, (n + 1) * TOKW - bn * S)
                qkvT_d = qkv_d[bn]
                xs = xp.tile([128, KT, TOKW], F32R, name="xs", tag="xs")
                nc.sync.dma_start(xs[:], xT_kp[:, :, sl])
                cs = tabp.tile([128, TOKW], F32R, name="cs", tag="cs")
                sn = tabp.tile([128, TOKW], F32R, name="sn", tag="sn")
                nc.sync.dma_start(cs[:], cos4[:, sl])
                nc.sync.dma_start(sn[:], sin4[:, sl])
                if n == 0:
                    for m in range(QKV_M):
                        nc.sync.dma_start(b_t[m][:], bias[m * 128:(m + 1) * 128, :])

                # k-outer / m-inner: compute starts as soon as w_t[0] + xs
                # arrive, all QKV_M psum banks accumulate in parallel.
                # Weight loads stream in during slice 0's compute.
                pss = [ps1.tile([128, TOKW], F32, name=f"qkps{m}", tag=f"qkps{m}")
                       for m in range(QKV_M)]
                for k in range(KT):
                    if n == 0:
                        nc.sync.dma_start(w_t[k][:], wT[k * 128:(k + 1) * 128, :])
                    for m in range(QKV_M):
                        nc.tensor.matmul(
                            pss[m][:], w_t[k][:, m * 128:(m + 1) * 128],
                            xs[:, k, :],
                            start=(k == 0), stop=(k == KT - 1))
                st = []
                for m in range(QKV_M):
                    s = stp.tile([128, TOKW], F32R, name=f"st{m}", tag=f"st{m}")
                    nc.scalar.activation(s[:], pss[m][:], Ident, bias=b_t[m][:])
                    st.append(s)

                # RoPE on q (m0 = packed x1 of all heads, m1 = packed x2)
                nh = NQH * 32
                o1 = ropep.tile([128, TOKW], F32R, name="o1", tag="o1")
                o2 = ropep.tile([128, TOKW], F32R, name="o2", tag="o2")
                sc1 = ropep.tile([128, TOKW], F32R, name="sc1", tag="sc1")
                nc.vector.tensor_mul(o1[:nh], st[0][:nh], cs[:nh])
                nc.vector.tensor_mul(sc1[:nh], st[1][:nh], sn[:nh])
                nc.vector.tensor_sub(o1[:nh], o1[:nh], sc1[:nh])
                nc.vector.tensor_mul(o2[:nh], st[1][:nh], cs[:nh])
                nc.vector.tensor_mul(sc1[:nh], st[0][:nh], sn[:nh])
                nc.vector.tensor_add(o2[:nh], o2[:nh], sc1[:nh])

                # RoPE on k (m = NQH tile: rows 0-31 x1, 32-63 x2, 64-127 pass).
                # DVE needs equal base partitions, so shift x2 down to rows
                # 0-31 first via SBUF->SBUF DMA, compute both halves at base 0.
                km = st[NQH]
                kx2 = ropep.tile([32, TOKW], F32R, name="kx2", tag="kx2")
                nc.sync.dma_start(kx2[:], km[32:64])
                ko1 = ropep.tile([32, TOKW], F32R, name="ko1", tag="ko1")
                ko2 = ropep.tile([32, TOKW], F32R, name="ko2", tag="ko2")
                sc2 = ropep.tile([32, TOKW], F32R, name="sc2", tag="sc2")
                nc.vector.tensor_mul(ko1[:], km[0:32], cs[0:32])
                nc.vector.tensor_mul(sc2[:], kx2[:], sn[0:32])
                nc.vector.tensor_sub(ko1[:], ko1[:], sc2[:])
                nc.vector.tensor_mul(ko2[:], kx2[:], cs[0:32])
                nc.vector.tensor_mul(sc2[:], km[0:32], sn[0:32])
                nc.vector.tensor_add(ko2[:], ko2[:], sc2[:])

                # spill q in packed block layout [x1 | x2 | pass0 | pass1],
                # each block nh rows; per-head assembly happens at reload
                nc.sync.dma_start(qkvT_d[0 * nh:1 * nh, osl], o1[:nh])
                nc.sync.dma_start(qkvT_d[1 * nh:2 * nh, osl], o2[:nh])
                nc.sync.dma_start(qkvT_d[2 * nh:3 * nh, osl], st[2][:nh])
                nc.sync.dma_start(qkvT_d[3 * nh:4 * nh, osl], st[3][:nh])
                nc.sync.dma_start(qkvT_d[QR:QR + 32, osl], ko1[:])
                nc.sync.dma_start(qkvT_d[QR + 32:QR + 64, osl], ko2[:])
                nc.sync.dma_start(qkvT_d[QR + 64:QR + 128, osl], km[64:128])
                nc.sync.dma_start(qkvT_d[QR + 128:QR + 256, osl], st[NQH + 1][:])

        # ---------------- phase 2: causal attention per (batch, head) -----
        with (
            tc.tile_pool(name="constp", bufs=1) as constp,
            tc.tile_pool(name="kvp", bufs=1) as kvp,
            tc.tile_pool(name="vnp", bufs=1) as vnp,
            tc.tile_pool(name="qp", bufs=2) as qp,
            tc.tile_pool(name="exp_", bufs=4) as exp_,
            tc.tile_pool(name="normp", bufs=1) as normp,
            tc.tile_pool(name="cup", bufs=1) as cup,
            tc.tile_pool(name="cnp", bufs=3) as cnp,
            tc.tile_pool(name="scps", bufs=4, space="PSUM") as scps,
            tc.tile_pool(name="ctxps", bufs=2, space="PSUM") as ctxps,
            tc.tile_pool(name="rsps", bufs=2, space="PSUM") as rsps,
        ):
            id_t = constp.tile([128, 128], F32R, name="id_t")
            on_t = constp.tile([128, 128], F32R, name="on_t")
            mk_t = constp.tile([128, 4 * QW], F32R, name="mk_t")
            nc.gpsimd.dma_start(id_t[:], ident[:, :])
            nc.gpsimd.dma_start(on_t[:], onesc[:, :])
            nc.gpsimd.dma_start(mk_t[:], maskt[:, :])

            for b in range(2):
                qkvT_d = qkv_d[b]
                kf = kvp.tile([128, S], F32R, name="kf", tag="kf")
                vT = kvp.tile([128, S], F32R, name="vT", tag="vT")
                nc.gpsimd.dma_start(kf[:], qkvT_d[QR:QR + 128, :])
                nc.gpsimd.dma_start(vT[:], qkvT_d[QR + 128:QR + 256, :])
                vn = []
                for j in range(NVT):
                    tp = scps.tile([128, 128], F32R, name="tp", tag="sc")
                    nc.tensor.transpose(tp[:], vT[:, j * 128:(j + 1) * 128], id_t[:])
                    v_j = vnp.tile([128, 128], F32R, name=f"vn{j}", tag=f"vn{j}")
                    nc.scalar.copy(v_j[:], tp[:])
                    vn.append(v_j)

                for h in range(NQH):
                    # assemble per-head q [x1, x2, pass0, pass1] from the
                    # packed block spill
                    nh = NQH * 32
                    qf = qp.tile([128, S], F32R, name="qf", tag="qf")
                    for blk in range(4):
                        src = blk * nh + h * 32
                        nc.gpsimd.dma_start(
                            qf[blk * 32:(blk + 1) * 32, :],
                            qkvT_d[src:src + 32, :])
                    recs, cus = [], []
                    for qt in range(NQT):
                        nkt = (qt * QW + QW) // 128
                        diag0 = qt * QW // 128
                        ctx_ps = ctxps.tile([128, QW], F32, name="ctx_ps", tag="ctx")
                        rs_ps = rsps.tile([1, QW], F32, name="rs_ps", tag="rs")
                        for kt in range(nkt):
                            sc_ps = scps.tile([128, QW], F32, name="sc_ps", tag="sc")
                            nc.tensor.matmul(
                                sc_ps[:], kf[:, kt * 128:(kt + 1) * 128],
                                qf[:, qt * QW:(qt + 1) * QW],
                                start=True, stop=True)
                            ex = exp_.tile([128, QW], F32R, name="ex", tag="ex")
                            nc.scalar.activation(ex[:], sc_ps[:], Exp, scale=scale)
                            if kt >= diag0:
                                d = kt - diag0
                                nc.vector.tensor_mul(
                                    ex[:], ex[:], mk_t[:, d * QW:(d + 1) * QW])
                            nc.tensor.matmul(ctx_ps[:], vn[kt][:], ex[:],
                                             start=(kt == 0), stop=(kt == nkt - 1))
                            nc.tensor.matmul(rs_ps[:], on_t[:, 0:1], ex[:],
                                             start=(kt == 0), stop=(kt == nkt - 1))
                        # off the PE critical path: reciprocal on DVE, ctx
                        # parked unnormalized in SBUF; normalization happens
                        # after all q-tiles of this head (PE broadcasts then
                        # run back-to-back with recips long since done)
                        rec = normp.tile([1, QW], F32R, name="rec", tag=f"rec{qt}")
                        with nc.allow_low_precision(reason="tf32 softmax denom"):
                            nc.vector.reciprocal(rec[:], rs_ps[0:1, :])
                        cu = cup.tile([128, QW], F32R, name="cu", tag=f"cu{qt}")
                        nc.scalar.copy(cu[:], ctx_ps[:])
                        recs.append(rec)
                        cus.append(cu)
                    for qt in range(NQT):
                        bc_ps = scps.tile([128, QW], F32, name="bc_ps", tag="sc")
                        nc.tensor.matmul(bc_ps[:], on_t[0:1, :], recs[qt][:],
                                         start=True, stop=True)
                        bc = normp.tile([128, QW], F32, name="bc", tag="bcs")
                        nc.vector.tensor_copy(bc[:], bc_ps[:])
                        cn = cnp.tile([128, QW], F32R, name="cn", tag="cn")
                        nc.vector.tensor_mul(cn[:], cus[qt][:], bc[:])
                        nc.sync.dma_start(
                            ctx_d[b][h * 128:(h + 1) * 128,
                                     qt * QW:(qt + 1) * QW],
                            cn[:])

        # ---------------- phase 3: dense partial outT = dwT.T @ ctxT ------
        # processed per batch half so half 0 can start while batch 1's
        # attention is still running
        with (
            tc.tile_pool(name="dwp", bufs=1) as dwp,
            tc.tile_pool(name="cxp", bufs=2) as cxp,
            tc.tile_pool(name="outp", bufs=4) as outp,
            tc.tile_pool(name="ps3", bufs=6, space="PSUM") as ps3,
        ):
            dw_t = [dwp.tile([128, H], F32R, name=f"dw{k}", tag=f"dw{k}")
                    for k in range(DKT)]
            for k in range(DKT):
                nc.gpsimd.dma_start(dw_t[k][:], dwT[k * 128:(k + 1) * 128, :])
            for half in range(2):
                cx_t = [cxp.tile([128, S], F32R, name=f"cx{k}", tag=f"cx{k}")
                        for k in range(DKT)]
                for k in range(DKT):
                    nc.gpsimd.dma_start(
                        cx_t[k][:], ctx_d[half][k * 128:(k + 1) * 128, :])
                for mo in range(H // 128):
                    for n in range(S // QW):
                        ps = ps3.tile([128, QW], F32, name="ps3t", tag="ps3t")
                        for k in range(DKT):
                            nc.tensor.matmul(
                                ps[:], dw_t[k][:, mo * 128:(mo + 1) * 128],
                                cx_t[k][:, n * QW:(n + 1) * QW],
                                start=(k == 0), stop=(k == DKT - 1))
                        ot = outp.tile([128, QW], F32, name="ot", tag="ot")
                        if (mo + n) % 2 == 0:
                            nc.scalar.copy(ot[:], ps[:])
                        else:
                            nc.vector.tensor_copy(ot[:], ps[:])
                        nc.sync.dma_start(
                            outT[mo * 128:(mo + 1) * 128,
                                 half * S + n * QW: half * S + (n + 1) * QW],
                            ot[:])

    nc.compile()
    return nc


# ---------------------------------------------------------------------------
# host side: sharding, tables, gather
# ---------------------------------------------------------------------------

def _head_perm(base):
    """Row order inside one head: interleaved-rotary x1, x2, then passthrough."""
    return (
        [base + 2 * r for r in range(32)]
        + [base + 2 * r + 1 for r in range(32)]
        + [base + d for d in range(ROT, HEAD_DIM)]
    )


def _core_inputs(core, hidden2d, qkv_w, qkv_b, dense_w, S, H, NQH, shared):
    n_heads = N_CORES * NQH
    heads_per_kv = n_heads // KV_HEADS
    kvh = (core * NQH) // heads_per_kv
    # packed phase-1 row order: m0 = all heads x1, m1 = x2, m2/m3 = pass,
    # m_NQH = k head [x1, x2, pass], m_NQH+1 = v head natural
    rows = []
    for blk in range(4):  # x1 | x2 | pass0 | pass1
        for h in range(NQH):
            base = (core * NQH + h) * HEAD_DIM
            p = _head_perm(base)
            rows.extend(p[blk * 32:(blk + 1) * 32])
    rows.extend(_head_perm(n_heads * HEAD_DIM + kvh * HEAD_DIM))
    vbase = n_heads * HEAD_DIM + KV_HEADS * HEAD_DIM + kvh * HEAD_DIM
    rows.extend(range(vbase, vbase + HEAD_DIM))
    rows = np.asarray(rows)

    w_shard = np.ascontiguousarray(qkv_w[rows].T)          # [H, (NQH+2)*128]
    b_shard = np.ascontiguousarray(qkv_b[rows][:, None])   # [(NQH+2)*128, 1]
    csl = slice(core * NQH * HEAD_DIM, (core + 1) * NQH * HEAD_DIM)
    dw_shard = np.ascontiguousarray(dense_w[:, csl].T)     # [NQH*128, H]
    return dict(
        xT=shared["xT"], wT=w_shard, bias=b_shard, dwT=dw_shard,
        cos4=shared["cos4"], sin4=shared["sin4"], maskt=shared["maskt"],
        ident=shared["ident"], onesc=shared["onesc"],
    )


def _shared_inputs(hidden2d, S):
    T = 2 * S
    xT = np.ascontiguousarray(hidden2d.T)                  # [H, T]
    inv = 1.0 / (ROPE_BASE ** (np.arange(0, ROT, 2, dtype=np.float64) / ROT))
    fr = np.arange(S, dtype=np.float64)[:, None] * inv[None, :]     # [S, 32]
    cosT = np.cos(fr).T.astype(np.float32)                 # [32, S]
    sinT = np.sin(fr).T.astype(np.float32)
    cos4 = np.ascontiguousarray(np.tile(np.tile(cosT, (4, 1)), (1, 2)))  # [128, T]
    sin4 = np.ascontiguousarray(np.tile(np.tile(sinT, (4, 1)), (1, 2)))
    j = np.arange(128)[:, None]
    i = np.arange(QW)[None, :]
    maskt = np.concatenate(
        [(j + d * 128 <= i).astype(np.float32) for d in range(4)], axis=1)  # [128, 4*QW]
    ident = np.eye(128, dtype=np.float32)
    onesc = np.ones((128, 128), dtype=np.float32)
    return dict(xT=xT, cos4=cos4, sin4=sin4, maskt=maskt, ident=ident, onesc=onesc)


def run(hidden_states, qkv_w, qkv_b, dense_w, S, H, NQH, trace=False):
    B = hidden_states.shape[0]
    T = B * S
    hidden2d = np.ascontiguousarray(
        hidden_states.reshape(T, H).astype(np.float32))
    shared = _shared_inputs(hidden2d, S)
    in_maps = [
        _core_inputs(c, hidden2d, qkv_w, qkv_b, dense_w, S, H, NQH, shared)
        for c in range(N_CORES)
    ]
    nc = build_core_kernel(S, H, NQH)
    res = run_bass_kernel_spmd(
        nc, in_maps, core_ids=list(range(N_CORES)), trace=trace)
    total = np.zeros((H, T), dtype=np.float64)
    for c in range(N_CORES):
        total += res.results[c]["outT"].astype(np.float64)
    out = total.T.astype(np.float32).reshape(B, S, H)
    return out, res


def kernel(hidden_states, qkv_w, qkv_b, dense_w):
    out, _ = run(
        np.asarray(hidden_states, dtype=np.float32),
        np.asarray(qkv_w, dtype=np.float32),
        np.asarray(qkv_b, dtype=np.float32),
        np.asarray(dense_w, dtype=np.float32),
        S=FULL["S"], H=FULL["H"], NQH=FULL["NQH"],
    )
    return out


# revision 22
# speedup vs baseline: 1.0611x; 1.0611x over previous
"""ChatGLM2 attention block (B=2, S=2048, H=4096, 32 q heads / 2 kv heads,
head_dim=128, partial interleaved RoPE) on 8 Trainium2 NeuronCores.

Sharding: tensor-parallel over heads. Core c owns q heads 4c..4c+3 and the
kv head c//4 (cores 0-3 -> kv0, 4-7 -> kv1). Each core computes its QKV
shard, causal attention for its 4 heads, and a partial dense projection
(contraction over its 512 ctx dims). Host sums the 8 partial outputs.

All matmuls run in float32r (TF32) at full PE rate. Everything on-device is
kept transposed ([feature, token]) so the PE contraction dim is always the
partition dim; the host transposes once at the end.
"""
import sys
import types

import numpy as np

sys.path.insert(0, "/opt/trn_rl_repo")

# NTFF profile hook (the image's antenv lacks axon_hooks; bass_utils wants it
# when trace=True). Registering it is harmless when tracing is off.
try:  # pragma: no cover - only matters when profiling
    import trn_agent_boot.trn_boot as _tb

    _m = types.ModuleType("antenv.axon_hooks")
    _hook = _tb._ntff_profile_via_ctypes("/opt/axon/libaxon_pjrt.so")
    _m.get_axon_ntff_profile_hook = lambda: _hook
    _m.set_axon_ntff_profile_hook = lambda h: None
    sys.modules.setdefault("antenv.axon_hooks", _m)
except Exception:
    pass

import concourse.bass as bass
import concourse.tile as tile
from concourse import bacc, mybir
from concourse.bass_utils import run_bass_kernel_spmd

F32 = mybir.dt.float32
F32R = mybir.dt.float32r

N_CORES = 8
HEAD_DIM = 128
ROT = 64          # rotary dims per head (first half, interleaved pairs)
KV_HEADS = 2
ROPE_BASE = 10000.0

# full-problem sizes
FULL = dict(B=2, S=2048, H=4096, NQH=4)

TOKW = 256        # phase-1 token slice width
QW = 512          # attention q-tile width / matmul free dim


def build_core_kernel(S, H, NQH, debug_spill=False):
    """One core's program. S = seq len per batch, H = hidden, NQH = q heads
    per core. T = 2*S tokens. Returns a compiled Bacc."""
    T = 2 * S
    KT = H // 128               # qkv contraction tiles
    QKV_M = NQH + 2             # per-core qkv output row tiles (q heads, k, v)
    QR = NQH * 128              # q rows
    NS = T // TOKW              # phase-1 token slices
    NQT = S // QW               # q tiles per batch
    NVT = S // 128              # v/k token tiles per batch
    DKT = NQH                   # dense contraction tiles (per-core ctx dims / 128)

    nc = bacc.Bacc("TRN2", target_bir_lowering=False, debug=False)

    xT = nc.dram_tensor("xT", [H, T], F32R, kind="ExternalInput")
    wT = nc.dram_tensor("wT", [H, QKV_M * 128], F32R, kind="ExternalInput")
    bias = nc.dram_tensor("bias", [QKV_M * 128, 1], F32, kind="ExternalInput")
    dwT = nc.dram_tensor("dwT", [NQH * 128, H], F32R, kind="ExternalInput")
    cos4 = nc.dram_tensor("cos4", [128, T], F32R, kind="ExternalInput")
    sin4 = nc.dram_tensor("sin4", [128, T], F32R, kind="ExternalInput")
    maskt = nc.dram_tensor("maskt", [128, 4 * QW], F32R, kind="ExternalInput")
    ident = nc.dram_tensor("ident", [128, 128], F32R, kind="ExternalInput")
    onesc = nc.dram_tensor("onesc", [128, 128], F32R, kind="ExternalInput")
    outT = nc.dram_tensor("outT", [H, T], F32, kind="ExternalOutput")

    # per-batch spill tensors: a reader of batch b's tensor only has to wait
    # for batch b's phase-1/2 writes, so phases overlap across batches
    dbg_kind = dict(kind="ExternalOutput") if debug_spill else {}
    qkv_d = [nc.dram_tensor(f"qkvT{b}_d", [QKV_M * 128, S], F32R, **dbg_kind)
             for b in range(2)]
    ctx_d = [nc.dram_tensor(f"ctxT{b}_d", [NQH * 128, S], F32R, **dbg_kind)
             for b in range(2)]

    scale = 1.0 / float(np.sqrt(HEAD_DIM))
    Exp = mybir.ActivationFunctionType.Exp
    Ident = mybir.ActivationFunctionType.Identity

    with tile.TileContext(nc) as tc:
        # ---------------- phase 1: qkvT = W @ xT (+bias, +RoPE), spill ----
        with (
            tc.tile_pool(name="wp", bufs=1) as wp,
            tc.tile_pool(name="biasp", bufs=1) as biasp,
            tc.tile_pool(name="xp", bufs=2) as xp,
            tc.tile_pool(name="tabp", bufs=2) as tabp,
            tc.tile_pool(name="stp", bufs=2) as stp,
            tc.tile_pool(name="ropep", bufs=2) as ropep,
            tc.tile_pool(name="ps1", bufs=1, space="PSUM") as ps1,
        ):
            w_t = [wp.tile([128, QKV_M * 128], F32R, name=f"w{k}", tag=f"w{k}")
                   for k in range(KT)]
            b_t = [biasp.tile([128, 1], F32, name=f"b{m}", tag=f"b{m}")
                   for m in range(QKV_M)]

            xT_kp = xT.rearrange("(k p) t -> p k t", p=128)

            for n in range(NS):
                sl = slice(n * TOKW, (n + 1) * TOKW)
                bn = (n * TOKW) // S
                osl = slice(n * TOKW - bn * S, (n + 1) * TOKW - bn * S)
                qkvT_d = qkv_d[bn]
                kh = KT // 2
                xs_a = xp.tile([128, kh, TOKW], F32R, name="xs_a", tag="xs_a")
                xs_b = xp.tile([128, KT - kh, TOKW], F32R, name="xs_b", tag="xs_b")
                nc.sync.dma_start(xs_a[:], xT_kp[:, 0:kh, sl])
                nc.sync.dma_start(xs_b[:], xT_kp[:, kh:KT, sl])
                cs = tabp.tile([128, TOKW], F32R, name="cs", tag="cs")
                sn = tabp.tile([128, TOKW], F32R, name="sn", tag="sn")
                nc.sync.dma_start(cs[:], cos4[:, sl])
                nc.sync.dma_start(sn[:], sin4[:, sl])
                if n == 0:
                    for m in range(QKV_M):
                        nc.sync.dma_start(b_t[m][:], bias[m * 128:(m + 1) * 128, :])

                # k-outer / m-inner: compute starts as soon as w_t[0] + xs
                # arrive, all QKV_M psum banks accumulate in parallel.
                # Weight loads stream in during slice 0's compute.
                pss = [ps1.tile([128, TOKW], F32, name=f"qkps{m}", tag=f"qkps{m}")
                       for m in range(QKV_M)]
                for k in range(KT):
                    if n == 0:
                        nc.sync.dma_start(w_t[k][:], wT[k * 128:(k + 1) * 128, :])
                    for m in range(QKV_M):
                        xsl = xs_a[:, k, :] if k < kh else xs_b[:, k - kh, :]
                        nc.tensor.matmul(
                            pss[m][:], w_t[k][:, m * 128:(m + 1) * 128],
                            xsl,
                            start=(k == 0), stop=(k == KT - 1))
                st = []
                for m in range(QKV_M):
                    s = stp.tile([128, TOKW], F32R, name=f"st{m}", tag=f"st{m}")
                    nc.scalar.activation(s[:], pss[m][:], Ident, bias=b_t[m][:])
                    st.append(s)

                # RoPE on q (m0 = packed x1 of all heads, m1 = packed x2)
                nh = NQH * 32
                o1 = ropep.tile([128, TOKW], F32R, name="o1", tag="o1")
                o2 = ropep.tile([128, TOKW], F32R, name="o2", tag="o2")
                sc1 = ropep.tile([128, TOKW], F32R, name="sc1", tag="sc1")
                nc.vector.tensor_mul(o1[:nh], st[0][:nh], cs[:nh])
                nc.vector.tensor_mul(sc1[:nh], st[1][:nh], sn[:nh])
                nc.vector.tensor_sub(o1[:nh], o1[:nh], sc1[:nh])
                nc.vector.tensor_mul(o2[:nh], st[1][:nh], cs[:nh])
                nc.vector.tensor_mul(sc1[:nh], st[0][:nh], sn[:nh])
                nc.vector.tensor_add(o2[:nh], o2[:nh], sc1[:nh])

                # RoPE on k (m = NQH tile: rows 0-31 x1, 32-63 x2, 64-127 pass).
                # DVE needs equal base partitions, so shift x2 down to rows
                # 0-31 first via SBUF->SBUF DMA, compute both halves at base 0.
                km = st[NQH]
                kx2 = ropep.tile([32, TOKW], F32R, name="kx2", tag="kx2")
                nc.sync.dma_start(kx2[:], km[32:64])
                ko1 = ropep.tile([32, TOKW], F32R, name="ko1", tag="ko1")
                ko2 = ropep.tile([32, TOKW], F32R, name="ko2", tag="ko2")
                sc2 = ropep.tile([32, TOKW], F32R, name="sc2", tag="sc2")
                nc.vector.tensor_mul(ko1[:], km[0:32], cs[0:32])
                nc.vector.tensor_mul(sc2[:], kx2[:], sn[0:32])
                nc.vector.tensor_sub(ko1[:], ko1[:], sc2[:])
                nc.vector.tensor_mul(ko2[:], kx2[:], cs[0:32])
                nc.vector.tensor_mul(sc2[:], km[0:32], sn[0:32])
                nc.vector.tensor_add(ko2[:], ko2[:], sc2[:])

                # spill q in packed block layout [x1 | x2 | pass0 | pass1],
                # each block nh rows; per-head assembly happens at reload
                nc.sync.dma_start(qkvT_d[0 * nh:1 * nh, osl], o1[:nh])
                nc.sync.dma_start(qkvT_d[1 * nh:2 * nh, osl], o2[:nh])
                nc.sync.dma_start(qkvT_d[2 * nh:3 * nh, osl], st[2][:nh])
                nc.sync.dma_start(qkvT_d[3 * nh:4 * nh, osl], st[3][:nh])
                nc.sync.dma_start(qkvT_d[QR:QR + 32, osl], ko1[:])
                nc.sync.dma_start(qkvT_d[QR + 32:QR + 64, osl], ko2[:])
                nc.sync.dma_start(qkvT_d[QR + 64:QR + 128, osl], km[64:128])
                nc.sync.dma_start(qkvT_d[QR + 128:QR + 256, osl], st[NQH + 1][:])

        # ---------------- phase 2: causal attention per (batch, head) -----
        with (
            tc.tile_pool(name="constp", bufs=1) as constp,
            tc.tile_pool(name="kvp", bufs=2) as kvp,
            tc.tile_pool(name="vnp", bufs=1) as vnp,
            tc.tile_pool(name="qp", bufs=2) as qp,
            tc.tile_pool(name="exp_", bufs=4) as exp_,
            tc.tile_pool(name="normp", bufs=1) as normp,
            tc.tile_pool(name="cup", bufs=1) as cup,
            tc.tile_pool(name="cnp", bufs=3) as cnp,
            tc.tile_pool(name="scps", bufs=4, space="PSUM") as scps,
            tc.tile_pool(name="ctxps", bufs=2, space="PSUM") as ctxps,
            tc.tile_pool(name="rsps", bufs=2, space="PSUM") as rsps,
        ):
            id_t = constp.tile([128, 128], F32R, name="id_t")
            on_t = constp.tile([128, 128], F32R, name="on_t")
            mk_t = constp.tile([128, 4 * QW], F32R, name="mk_t")
            nc.gpsimd.dma_start(id_t[:], ident[:, :])
            nc.gpsimd.dma_start(on_t[:], onesc[:, :])
            nc.gpsimd.dma_start(mk_t[:], maskt[:, :])

            for b in range(2):
                qkvT_d = qkv_d[b]
                kf = kvp.tile([128, S], F32R, name="kf", tag="kf")
                vT = kvp.tile([128, S], F32R, name="vT", tag="vT")
                nc.gpsimd.dma_start(kf[:], qkvT_d[QR:QR + 128, :])
                nc.gpsimd.dma_start(vT[:], qkvT_d[QR + 128:QR + 256, :])
                vn = []
                for j in range(NVT):
                    tp = scps.tile([128, 128], F32R, name="tp", tag="sc")
                    nc.tensor.transpose(tp[:], vT[:, j * 128:(j + 1) * 128], id_t[:])
                    v_j = vnp.tile([128, 128], F32R, name=f"vn{j}", tag=f"vn{j}")
                    nc.scalar.copy(v_j[:], tp[:])
                    vn.append(v_j)

                for h in range(NQH):
                    # assemble per-head q [x1, x2, pass0, pass1] from the
                    # packed block spill
                    nh = NQH * 32
                    qf = qp.tile([128, S], F32R, name="qf", tag="qf")
                    for blk in range(4):
                        src = blk * nh + h * 32
                        nc.sync.dma_start(
                            qf[blk * 32:(blk + 1) * 32, :],
                            qkvT_d[src:src + 32, :])
                    recs, cus = [], []
                    for qt in range(NQT):
                        nkt = (qt * QW + QW) // 128
                        diag0 = qt * QW // 128
                        ctx_ps = ctxps.tile([128, QW], F32, name="ctx_ps", tag="ctx")
                        rs_ps = rsps.tile([1, QW], F32, name="rs_ps", tag="rs")
                        for kt in range(nkt):
                            sc_ps = scps.tile([128, QW], F32, name="sc_ps", tag="sc")
                            nc.tensor.matmul(
                                sc_ps[:], kf[:, kt * 128:(kt + 1) * 128],
                                qf[:, qt * QW:(qt + 1) * QW],
                                start=True, stop=True)
                            ex = exp_.tile([128, QW], F32R, name="ex", tag="ex")
                            nc.scalar.activation(ex[:], sc_ps[:], Exp, scale=scale)
                            if kt >= diag0:
                                d = kt - diag0
                                nc.vector.tensor_mul(
                                    ex[:], ex[:], mk_t[:, d * QW:(d + 1) * QW])
                            nc.tensor.matmul(ctx_ps[:], vn[kt][:], ex[:],
                                             start=(kt == 0), stop=(kt == nkt - 1))
                            nc.tensor.matmul(rs_ps[:], on_t[:, 0:1], ex[:],
                                             start=(kt == 0), stop=(kt == nkt - 1))
                        # off the PE critical path: reciprocal on DVE, ctx
                        # parked unnormalized in SBUF; normalization happens
                        # after all q-tiles of this head (PE broadcasts then
                        # run back-to-back with recips long since done)
                        rec = normp.tile([1, QW], F32R, name="rec", tag=f"rec{qt}")
                        with nc.allow_low_precision(reason="tf32 softmax denom"):
                            nc.vector.reciprocal(rec[:], rs_ps[0:1, :])
                        cu = cup.tile([128, QW], F32R, name="cu", tag=f"cu{qt}")
                        nc.scalar.copy(cu[:], ctx_ps[:])
                        recs.append(rec)
                        cus.append(cu)
                    for qt in range(NQT):
                        bc_ps = scps.tile([128, QW], F32, name="bc_ps", tag="sc")
                        nc.tensor.matmul(bc_ps[:], on_t[0:1, :], recs[qt][:],
                                         start=True, stop=True)
                        bc = normp.tile([128, QW], F32, name="bc", tag="bcs")
                        nc.vector.tensor_copy(bc[:], bc_ps[:])
                        cn = cnp.tile([128, QW], F32R, name="cn", tag="cn")
                        nc.vector.tensor_mul(cn[:], cus[qt][:], bc[:])
                        nc.sync.dma_start(
                            ctx_d[b][h * 128:(h + 1) * 128,
                                     qt * QW:(qt + 1) * QW],
                            cn[:])

        # ---------------- phase 3: dense partial outT = dwT.T @ ctxT ------
        # processed per batch half so half 0 can start while batch 1's
        # attention is still running
        with (
            tc.tile_pool(name="dwp", bufs=1) as dwp,
            tc.tile_pool(name="cxp", bufs=2) as cxp,
            tc.tile_pool(name="outp", bufs=4) as outp,
            tc.tile_pool(name="ps3", bufs=6, space="PSUM") as ps3,
        ):
            dw_t = [dwp.tile([128, H], F32R, name=f"dw{k}", tag=f"dw{k}")
                    for k in range(DKT)]
            for k in range(DKT):
                nc.scalar.dma_start(dw_t[k][:], dwT[k * 128:(k + 1) * 128, :])
            for half in range(2):
                cx_t = [cxp.tile([128, S], F32R, name=f"cx{k}", tag=f"cx{k}")
                        for k in range(DKT)]
                for k in range(DKT):
                    nc.scalar.dma_start(
                        cx_t[k][:], ctx_d[half][k * 128:(k + 1) * 128, :])
                for mo in range(H // 128):
                    for n in range(S // QW):
                        ps = ps3.tile([128, QW], F32, name="ps3t", tag="ps3t")
                        for k in range(DKT):
                            nc.tensor.matmul(
                                ps[:], dw_t[k][:, mo * 128:(mo + 1) * 128],
                                cx_t[k][:, n * QW:(n + 1) * QW],
                                start=(k == 0), stop=(k == DKT - 1))
                        ot = outp.tile([128, QW], F32, name="ot", tag="ot")
                        if (mo + n) % 2 == 0:
                            nc.scalar.copy(ot[:], ps[:])
                        else:
                            nc.vector.tensor_copy(ot[:], ps[:])
                        nc.sync.dma_start(
                            outT[mo * 128:(mo + 1) * 128,
                                 half * S + n * QW: half * S + (n + 1) * QW],
                            ot[:])

    nc.compile()
    return nc


# ---------------------------------------------------------------------------
# host side: sharding, tables, gather
# ---------------------------------------------------------------------------

def _head_perm(base):
    """Row order inside one head: interleaved-rotary x1, x2, then passthrough."""
    return (
        [base + 2 * r for r in range(32)]
        + [base + 2 * r + 1 for r in range(32)]
        + [base + d for d in range(ROT, HEAD_DIM)]
    )


def _core_inputs(core, hidden2d, qkv_w, qkv_b, dense_w, S, H, NQH, shared):
    n_heads = N_CORES * NQH
    heads_per_kv = n_heads // KV_HEADS
    kvh = (core * NQH) // heads_per_kv
    # packed phase-1 row order: m0 = all heads x1, m1 = x2, m2/m3 = pass,
    # m_NQH = k head [x1, x2, pass], m_NQH+1 = v head natural
    rows = []
    for blk in range(4):  # x1 | x2 | pass0 | pass1
        for h in range(NQH):
            base = (core * NQH + h) * HEAD_DIM
            p = _head_perm(base)
            rows.extend(p[blk * 32:(blk + 1) * 32])
    rows.extend(_head_perm(n_heads * HEAD_DIM + kvh * HEAD_DIM))
    vbase = n_heads * HEAD_DIM + KV_HEADS * HEAD_DIM + kvh * HEAD_DIM
    rows.extend(range(vbase, vbase + HEAD_DIM))
    rows = np.asarray(rows)

    w_shard = np.ascontiguousarray(qkv_w[rows].T)          # [H, (NQH+2)*128]
    b_shard = np.ascontiguousarray(qkv_b[rows][:, None])   # [(NQH+2)*128, 1]
    csl = slice(core * NQH * HEAD_DIM, (core + 1) * NQH * HEAD_DIM)
    dw_shard = np.ascontiguousarray(dense_w[:, csl].T)     # [NQH*128, H]
    return dict(
        xT=shared["xT"], wT=w_shard, bias=b_shard, dwT=dw_shard,
        cos4=shared["cos4"], sin4=shared["sin4"], maskt=shared["maskt"],
        ident=shared["ident"], onesc=shared["onesc"],
    )


def _shared_inputs(hidden2d, S):
    T = 2 * S
    xT = np.ascontiguousarray(hidden2d.T)                  # [H, T]
    inv = 1.0 / (ROPE_BASE ** (np.arange(0, ROT, 2, dtype=np.float64) / ROT))
    fr = np.arange(S, dtype=np.float64)[:, None] * inv[None, :]     # [S, 32]
    cosT = np.cos(fr).T.astype(np.float32)                 # [32, S]
    sinT = np.sin(fr).T.astype(np.float32)
    cos4 = np.ascontiguousarray(np.tile(np.tile(cosT, (4, 1)), (1, 2)))  # [128, T]
    sin4 = np.ascontiguousarray(np.tile(np.tile(sinT, (4, 1)), (1, 2)))
    j = np.arange(128)[:, None]
    i = np.arange(QW)[None, :]
    maskt = np.concatenate(
        [(j + d * 128 <= i).astype(np.float32) for d in range(4)], axis=1)  # [128, 4*QW]
    ident = np.eye(128, dtype=np.float32)
    onesc = np.ones((128, 128), dtype=np.float32)
    return dict(xT=xT, cos4=cos4, sin4=sin4, maskt=maskt, ident=ident, onesc=onesc)


def run(hidden_states, qkv_w, qkv_b, dense_w, S, H, NQH, trace=False):
    B = hidden_states.shape[0]
    T = B * S
    hidden2d = np.ascontiguousarray(
        hidden_states.reshape(T, H).astype(np.float32))
    shared = _shared_inputs(hidden2d, S)
    in_maps = [
        _core_inputs(c, hidden2d, qkv_w, qkv_b, dense_w, S, H, NQH, shared)
        for c in range(N_CORES)
    ]
    nc = build_core_kernel(S, H, NQH)
    res = run_bass_kernel_spmd(
        nc, in_maps, core_ids=list(range(N_CORES)), trace=trace)
    total = np.zeros((H, T), dtype=np.float64)
    for c in range(N_CORES):
        total += res.results[c]["outT"].astype(np.float64)
    out = total.T.astype(np.float32).reshape(B, S, H)
    return out, res


def kernel(hidden_states, qkv_w, qkv_b, dense_w):
    out, _ = run(
        np.asarray(hidden_states, dtype=np.float32),
        np.asarray(qkv_w, dtype=np.float32),
        np.asarray(qkv_b, dtype=np.float32),
        np.asarray(dense_w, dtype=np.float32),
        S=FULL["S"], H=FULL["H"], NQH=FULL["NQH"],
    )
    return out


# revision 25
# speedup vs baseline: 1.0637x; 1.0025x over previous
"""ChatGLM2 attention block (B=2, S=2048, H=4096, 32 q heads / 2 kv heads,
head_dim=128, partial interleaved RoPE) on 8 Trainium2 NeuronCores.

Sharding: tensor-parallel over heads. Core c owns q heads 4c..4c+3 and the
kv head c//4 (cores 0-3 -> kv0, 4-7 -> kv1). Each core computes its QKV
shard, causal attention for its 4 heads, and a partial dense projection
(contraction over its 512 ctx dims). Host sums the 8 partial outputs.

All matmuls run in float32r (TF32) at full PE rate. Everything on-device is
kept transposed ([feature, token]) so the PE contraction dim is always the
partition dim; the host transposes once at the end.
"""
import sys
import types

import numpy as np

sys.path.insert(0, "/opt/trn_rl_repo")

# NTFF profile hook (the image's antenv lacks axon_hooks; bass_utils wants it
# when trace=True). Registering it is harmless when tracing is off.
try:  # pragma: no cover - only matters when profiling
    import trn_agent_boot.trn_boot as _tb

    _m = types.ModuleType("antenv.axon_hooks")
    _hook = _tb._ntff_profile_via_ctypes("/opt/axon/libaxon_pjrt.so")
    _m.get_axon_ntff_profile_hook = lambda: _hook
    _m.set_axon_ntff_profile_hook = lambda h: None
    sys.modules.setdefault("antenv.axon_hooks", _m)
except Exception:
    pass

import concourse.bass as bass
import concourse.tile as tile
from concourse import bacc, mybir
from concourse.bass_utils import run_bass_kernel_spmd

F32 = mybir.dt.float32
F32R = mybir.dt.float32r

N_CORES = 8
HEAD_DIM = 128
ROT = 64          # rotary dims per head (first half, interleaved pairs)
KV_HEADS = 2
ROPE_BASE = 10000.0

# full-problem sizes
FULL = dict(B=2, S=2048, H=4096, NQH=4)

TOKW = 256        # phase-1 token slice width
QW = 512          # attention q-tile width / matmul free dim


def build_core_kernel(S, H, NQH, debug_spill=False):
    """One core's program. S = seq len per batch, H = hidden, NQH = q heads
    per core. T = 2*S tokens. Returns a compiled Bacc."""
    T = 2 * S
    KT = H // 128               # qkv contraction tiles
    QKV_M = NQH + 2             # per-core qkv output row tiles (q heads, k, v)
    QR = NQH * 128              # q rows
    NS = T // TOKW              # phase-1 token slices
    NQT = S // QW               # q tiles per batch
    NVT = S // 128              # v/k token tiles per batch
    DKT = NQH                   # dense contraction tiles (per-core ctx dims / 128)

    nc = bacc.Bacc("TRN2", target_bir_lowering=False, debug=False)

    xT = nc.dram_tensor("xT", [H, T], F32R, kind="ExternalInput")
    wT = nc.dram_tensor("wT", [H, QKV_M * 128], F32R, kind="ExternalInput")
    bias = nc.dram_tensor("bias", [QKV_M * 128, 1], F32, kind="ExternalInput")
    dwT = nc.dram_tensor("dwT", [NQH * 128, H], F32R, kind="ExternalInput")
    cos4 = nc.dram_tensor("cos4", [128, T], F32R, kind="ExternalInput")
    sin4 = nc.dram_tensor("sin4", [128, T], F32R, kind="ExternalInput")
    maskt = nc.dram_tensor("maskt", [128, 4 * QW], F32R, kind="ExternalInput")
    ident = nc.dram_tensor("ident", [128, 128], F32R, kind="ExternalInput")
    onesc = nc.dram_tensor("onesc", [128, 128], F32R, kind="ExternalInput")
    outT = nc.dram_tensor("outT", [H, T], F32, kind="ExternalOutput")

    # per-batch spill tensors: a reader of batch b's tensor only has to wait
    # for batch b's phase-1/2 writes, so phases overlap across batches
    dbg_kind = dict(kind="ExternalOutput") if debug_spill else {}
    qkv_d = [nc.dram_tensor(f"qkvT{b}_d", [QKV_M * 128, S], F32R, **dbg_kind)
             for b in range(2)]
    ctx_d = [nc.dram_tensor(f"ctxT{b}_d", [NQH * 128, S], F32R, **dbg_kind)
             for b in range(2)]

    scale = 1.0 / float(np.sqrt(HEAD_DIM))
    Exp = mybir.ActivationFunctionType.Exp
    Ident = mybir.ActivationFunctionType.Identity

    with tile.TileContext(nc) as tc:
        from contextlib import ExitStack as _ES
        # ---------------- phase 1: qkvT = W @ xT (+bias, +RoPE), spill ----
        with (
            tc.tile_pool(name="wp", bufs=1) as wp,
            tc.tile_pool(name="biasp", bufs=1) as biasp,
            tc.tile_pool(name="xp", bufs=2) as xp,
            tc.tile_pool(name="tabp", bufs=2) as tabp,
            tc.tile_pool(name="stp", bufs=2) as stp,
            tc.tile_pool(name="ropep", bufs=2) as ropep,
            tc.tile_pool(name="ps1", bufs=1, space="PSUM") as ps1,
        ):
            w_t = [wp.tile([128, QKV_M * 128], F32R, name=f"w{k}", tag=f"w{k}")
                   for k in range(KT)]
            b_t = [biasp.tile([128, 1], F32, name=f"b{m}", tag=f"b{m}")
                   for m in range(QKV_M)]

            xT_kp = xT.rearrange("(k p) t -> p k t", p=128)

            _ph1 = nc.named_scope("ph1"); _ph1.__enter__()
            for n in range(NS):
                sl = slice(n * TOKW, (n + 1) * TOKW)
                bn = (n * TOKW) // S
                osl = slice(n * TOKW - bn * S, (n + 1) * TOKW - bn * S)
                qkvT_d = qkv_d[bn]
                kh = KT // 2
                xs_a = xp.tile([128, kh, TOKW], F32R, name="xs_a", tag="xs_a")
                xs_b = xp.tile([128, KT - kh, TOKW], F32R, name="xs_b", tag="xs_b")
                nc.sync.dma_start(xs_a[:], xT_kp[:, 0:kh, sl])
                nc.sync.dma_start(xs_b[:], xT_kp[:, kh:KT, sl])
                cs = tabp.tile([128, TOKW], F32R, name="cs", tag="cs")
                sn = tabp.tile([128, TOKW], F32R, name="sn", tag="sn")
                nc.sync.dma_start(cs[:], cos4[:, sl])
                nc.sync.dma_start(sn[:], sin4[:, sl])
                if n == 0:
                    for m in range(QKV_M):
                        nc.sync.dma_start(b_t[m][:], bias[m * 128:(m + 1) * 128, :])

                # k-outer / m-inner: compute starts as soon as w_t[0] + xs
                # arrive, all QKV_M psum banks accumulate in parallel.
                # Weight loads stream in during slice 0's compute.
                pss = [ps1.tile([128, TOKW], F32, name=f"qkps{m}", tag=f"qkps{m}")
                       for m in range(QKV_M)]
                for k in range(KT):
                    if n == 0:
                        nc.sync.dma_start(w_t[k][:], wT[k * 128:(k + 1) * 128, :])
                    for m in range(QKV_M):
                        xsl = xs_a[:, k, :] if k < kh else xs_b[:, k - kh, :]
                        nc.tensor.matmul(
                            pss[m][:], w_t[k][:, m * 128:(m + 1) * 128],
                            xsl,
                            start=(k == 0), stop=(k == KT - 1))
                st = []
                for m in range(QKV_M):
                    s = stp.tile([128, TOKW], F32R, name=f"st{m}", tag=f"st{m}")
                    nc.scalar.activation(s[:], pss[m][:], Ident, bias=b_t[m][:])
                    st.append(s)

                # RoPE on q (m0 = packed x1 of all heads, m1 = packed x2)
                nh = NQH * 32
                o1 = ropep.tile([128, TOKW], F32R, name="o1", tag="o1")
                o2 = ropep.tile([128, TOKW], F32R, name="o2", tag="o2")
                sc1 = ropep.tile([128, TOKW], F32R, name="sc1", tag="sc1")
                nc.vector.tensor_mul(o1[:nh], st[0][:nh], cs[:nh])
                nc.vector.tensor_mul(sc1[:nh], st[1][:nh], sn[:nh])
                nc.vector.tensor_sub(o1[:nh], o1[:nh], sc1[:nh])
                nc.vector.tensor_mul(o2[:nh], st[1][:nh], cs[:nh])
                nc.vector.tensor_mul(sc1[:nh], st[0][:nh], sn[:nh])
                nc.vector.tensor_add(o2[:nh], o2[:nh], sc1[:nh])

                # RoPE on k (m = NQH tile: rows 0-31 x1, 32-63 x2, 64-127 pass).
                # DVE needs equal base partitions, so shift x2 down to rows
                # 0-31 first via SBUF->SBUF DMA, compute both halves at base 0.
                km = st[NQH]
                kx2 = ropep.tile([32, TOKW], F32R, name="kx2", tag="kx2")
                nc.sync.dma_start(kx2[:], km[32:64])
                ko1 = ropep.tile([32, TOKW], F32R, name="ko1", tag="ko1")
                ko2 = ropep.tile([32, TOKW], F32R, name="ko2", tag="ko2")
                sc2 = ropep.tile([32, TOKW], F32R, name="sc2", tag="sc2")
                nc.vector.tensor_mul(ko1[:], km[0:32], cs[0:32])
                nc.vector.tensor_mul(sc2[:], kx2[:], sn[0:32])
                nc.vector.tensor_sub(ko1[:], ko1[:], sc2[:])
                nc.vector.tensor_mul(ko2[:], kx2[:], cs[0:32])
                nc.vector.tensor_mul(sc2[:], km[0:32], sn[0:32])
                nc.vector.tensor_add(ko2[:], ko2[:], sc2[:])

                # spill q in packed block layout [x1 | x2 | pass0 | pass1],
                # each block nh rows; per-head assembly happens at reload
                nc.sync.dma_start(qkvT_d[0 * nh:1 * nh, osl], o1[:nh])
                nc.sync.dma_start(qkvT_d[1 * nh:2 * nh, osl], o2[:nh])
                nc.sync.dma_start(qkvT_d[2 * nh:3 * nh, osl], st[2][:nh])
                nc.sync.dma_start(qkvT_d[3 * nh:4 * nh, osl], st[3][:nh])
                nc.sync.dma_start(qkvT_d[QR:QR + 32, osl], ko1[:])
                nc.sync.dma_start(qkvT_d[QR + 32:QR + 64, osl], ko2[:])
                nc.sync.dma_start(qkvT_d[QR + 64:QR + 128, osl], km[64:128])
                nc.sync.dma_start(qkvT_d[QR + 128:QR + 256, osl], st[NQH + 1][:])

        _ph1.__exit__(None, None, None)

        # ---------------- phase 2: causal attention per (batch, head) -----
        with (
            tc.tile_pool(name="constp", bufs=1) as constp,
            tc.tile_pool(name="kvp", bufs=2) as kvp,
            tc.tile_pool(name="vnp", bufs=1) as vnp,
            tc.tile_pool(name="qp", bufs=3) as qp,
            tc.tile_pool(name="exp_", bufs=4) as exp_,
            tc.tile_pool(name="normp", bufs=2) as normp,
            tc.tile_pool(name="cnp", bufs=3) as cnp,
            tc.tile_pool(name="scps", bufs=4, space="PSUM") as scps,
            tc.tile_pool(name="ctxps", bufs=2, space="PSUM") as ctxps,
            tc.tile_pool(name="rsps", bufs=2, space="PSUM") as rsps,
        ):
            id_t = constp.tile([128, 128], F32R, name="id_t")
            on_t = constp.tile([128, 128], F32R, name="on_t")
            mk_t = constp.tile([128, 4 * QW], F32R, name="mk_t")
            nc.gpsimd.dma_start(id_t[:], ident[:, :])
            nc.gpsimd.dma_start(on_t[:], onesc[:, :])
            nc.gpsimd.dma_start(mk_t[:], maskt[:, :])

            for b in range(2):
                _ph2 = nc.named_scope(f"ph2b{b}"); _ph2.__enter__()
                qkvT_d = qkv_d[b]
                kf = kvp.tile([128, S], F32R, name="kf", tag="kf")
                vT = kvp.tile([128, S], F32R, name="vT", tag="vT")
                nc.gpsimd.dma_start(kf[:], qkvT_d[QR:QR + 128, :])
                nc.gpsimd.dma_start(vT[:], qkvT_d[QR + 128:QR + 256, :])
                vn = []
                for j in range(NVT):
                    tp = scps.tile([128, 128], F32R, name="tp", tag="sc")
                    nc.tensor.transpose(tp[:], vT[:, j * 128:(j + 1) * 128], id_t[:])
                    v_j = vnp.tile([128, 128], F32R, name=f"vn{j}", tag=f"vn{j}")
                    nc.scalar.copy(v_j[:], tp[:])
                    vn.append(v_j)

                for h in range(NQH):
                    # assemble per-head q [x1, x2, pass0, pass1] from the
                    # packed block spill
                    nh = NQH * 32
                    qf = qp.tile([128, S], F32R, name="qf", tag="qf")
                    for blk in range(4):
                        src = blk * nh + h * 32
                        nc.sync.dma_start(
                            qf[blk * 32:(blk + 1) * 32, :],
                            qkvT_d[src:src + 32, :])
                    for qt in range(NQT):
                        nkt = (qt * QW + QW) // 128
                        diag0 = qt * QW // 128
                        ctx_ps = ctxps.tile([128, QW], F32, name="ctx_ps", tag="ctx")
                        rs_ps = rsps.tile([128, QW], F32, name="rs_ps", tag="rs")
                        for kt in range(nkt):
                            sc_ps = scps.tile([128, QW], F32, name="sc_ps", tag="sc")
                            nc.tensor.matmul(
                                sc_ps[:], kf[:, kt * 128:(kt + 1) * 128],
                                qf[:, qt * QW:(qt + 1) * QW],
                                start=True, stop=True)
                            ex = exp_.tile([128, QW], F32R, name="ex", tag="ex")
                            nc.scalar.activation(ex[:], sc_ps[:], Exp, scale=scale)
                            if kt >= diag0:
                                d = kt - diag0
                                nc.vector.tensor_mul(
                                    ex[:], ex[:], mk_t[:, d * QW:(d + 1) * QW])
                            nc.tensor.matmul(ctx_ps[:], vn[kt][:], ex[:],
                                             start=(kt == 0), stop=(kt == nkt - 1))
                            # full-ones stationary: every psum row gets the
                            # k-sum, i.e. the row-sum already broadcast
                            nc.tensor.matmul(rs_ps[:], on_t[:, :], ex[:],
                                             start=(kt == 0), stop=(kt == nkt - 1))
                        # normalization entirely off the PE: recip + mul on DVE
                        rec = normp.tile([128, QW], F32R, name="rec", tag="rec")
                        with nc.allow_low_precision(reason="tf32 softmax denom"):
                            nc.vector.reciprocal(rec[:], rs_ps[:])
                        cn = cnp.tile([128, QW], F32R, name="cn", tag="cn")
                        nc.vector.tensor_mul(cn[:], ctx_ps[:], rec[:])
                        nc.scalar.dma_start(
                            ctx_d[b][h * 128:(h + 1) * 128,
                                     qt * QW:(qt + 1) * QW],
                            cn[:])
                _ph2.__exit__(None, None, None)

        # ---------------- phase 3: dense partial outT = dwT.T @ ctxT ------
        # processed per batch half so half 0 can start while batch 1's
        # attention is still running
        with (
            tc.tile_pool(name="dwp", bufs=1) as dwp,
            tc.tile_pool(name="cxp", bufs=2) as cxp,
            tc.tile_pool(name="outp", bufs=4) as outp,
            tc.tile_pool(name="ps3", bufs=6, space="PSUM") as ps3,
        ):
            dw_t = [dwp.tile([128, H], F32R, name=f"dw{k}", tag=f"dw{k}")
                    for k in range(DKT)]
            for k in range(DKT):
                nc.gpsimd.dma_start(dw_t[k][:], dwT[k * 128:(k + 1) * 128, :])
            for half in range(2):
                _ph3 = nc.named_scope(f"ph3h{half}"); _ph3.__enter__()
                cx_t = [cxp.tile([128, S], F32R, name=f"cx{k}", tag=f"cx{k}")
                        for k in range(DKT)]
                for k in range(DKT):
                    nc.gpsimd.dma_start(
                        cx_t[k][:], ctx_d[half][k * 128:(k + 1) * 128, :])
                for mo in range(H // 128):
                    for n in range(S // QW):
                        ps = ps3.tile([128, QW], F32, name="ps3t", tag="ps3t")
                        for k in range(DKT):
                            nc.tensor.matmul(
                                ps[:], dw_t[k][:, mo * 128:(mo + 1) * 128],
                                cx_t[k][:, n * QW:(n + 1) * QW],
                                start=(k == 0), stop=(k == DKT - 1))
                        ot = outp.tile([128, QW], F32, name="ot", tag="ot")
                        if (mo + n) % 2 == 0:
                            nc.scalar.copy(ot[:], ps[:])
                        else:
                            nc.vector.tensor_copy(ot[:], ps[:])
                        nc.sync.dma_start(
                            outT[mo * 128:(mo + 1) * 128,
                                 half * S + n * QW: half * S + (n + 1) * QW],
                            ot[:])
                _ph3.__exit__(None, None, None)

    nc.compile()
    return nc


# ---------------------------------------------------------------------------
# host side: sharding, tables, gather
# ---------------------------------------------------------------------------

def _head_perm(base):
    """Row order inside one head: interleaved-rotary x1, x2, then passthrough."""
    return (
        [base + 2 * r for r in range(32)]
        + [base + 2 * r + 1 for r in range(32)]
        + [base + d for d in range(ROT, HEAD_DIM)]
    )


def _core_inputs(core, hidden2d, qkv_w, qkv_b, dense_w, S, H, NQH, shared):
    n_heads = N_CORES * NQH
    heads_per_kv = n_heads // KV_HEADS
    kvh = (core * NQH) // heads_per_kv
    # packed phase-1 row order: m0 = all heads x1, m1 = x2, m2/m3 = pass,
    # m_NQH = k head [x1, x2, pass], m_NQH+1 = v head natural
    rows = []
    for blk in range(4):  # x1 | x2 | pass0 | pass1
        for h in range(NQH):
            base = (core * NQH + h) * HEAD_DIM
            p = _head_perm(base)
            rows.extend(p[blk * 32:(blk + 1) * 32])
    rows.extend(_head_perm(n_heads * HEAD_DIM + kvh * HEAD_DIM))
    vbase = n_heads * HEAD_DIM + KV_HEADS * HEAD_DIM + kvh * HEAD_DIM
    rows.extend(range(vbase, vbase + HEAD_DIM))
    rows = np.asarray(rows)

    w_shard = np.ascontiguousarray(qkv_w[rows].T)          # [H, (NQH+2)*128]
    b_shard = np.ascontiguousarray(qkv_b[rows][:, None])   # [(NQH+2)*128, 1]
    csl = slice(core * NQH * HEAD_DIM, (core + 1) * NQH * HEAD_DIM)
    dw_shard = np.ascontiguousarray(dense_w[:, csl].T)     # [NQH*128, H]
    return dict(
        xT=shared["xT"], wT=w_shard, bias=b_shard, dwT=dw_shard,
        cos4=shared["cos4"], sin4=shared["sin4"], maskt=shared["maskt"],
        ident=shared["ident"], onesc=shared["onesc"],
    )


def _shared_inputs(hidden2d, S):
    T = 2 * S
    xT = np.ascontiguousarray(hidden2d.T)                  # [H, T]
    inv = 1.0 / (ROPE_BASE ** (np.arange(0, ROT, 2, dtype=np.float64) / ROT))
    fr = np.arange(S, dtype=np.float64)[:, None] * inv[None, :]     # [S, 32]
    cosT = np.cos(fr).T.astype(np.float32)                 # [32, S]
    sinT = np.sin(fr).T.astype(np.float32)
    cos4 = np.ascontiguousarray(np.tile(np.tile(cosT, (4, 1)), (1, 2)))  # [128, T]
    sin4 = np.ascontiguousarray(np.tile(np.tile(sinT, (4, 1)), (1, 2)))
    j = np.arange(128)[:, None]
    i = np.arange(QW)[None, :]
    maskt = np.concatenate(
        [(j + d * 128 <= i).astype(np.float32) for d in range(4)], axis=1)  # [128, 4*QW]
    ident = np.eye(128, dtype=np.float32)
    onesc = np.ones((128, 128), dtype=np.float32)
    return dict(xT=xT, cos4=cos4, sin4=sin4, maskt=maskt, ident=ident, onesc=onesc)


def run(hidden_states, qkv_w, qkv_b, dense_w, S, H, NQH, trace=False):
    B = hidden_states.shape[0]
    T = B * S
    hidden2d = np.ascontiguousarray(
        hidden_states.reshape(T, H).astype(np.float32))
    shared = _shared_inputs(hidden2d, S)
    in_maps = [
        _core_inputs(c, hidden2d, qkv_w, qkv_b, dense_w, S, H, NQH, shared)
        for c in range(N_CORES)
    ]
    nc = build_core_kernel(S, H, NQH)
    res = run_bass_kernel_spmd(
        nc, in_maps, core_ids=list(range(N_CORES)), trace=trace)
    total = np.zeros((H, T), dtype=np.float64)
    for c in range(N_CORES):
        total += res.results[c]["outT"].astype(np.float64)
    out = total.T.astype(np.float32).reshape(B, S, H)
    return out, res


def kernel(hidden_states, qkv_w, qkv_b, dense_w):
    out, _ = run(
        np.asarray(hidden_states, dtype=np.float32),
        np.asarray(qkv_w, dtype=np.float32),
        np.asarray(qkv_b, dtype=np.float32),
        np.asarray(dense_w, dtype=np.float32),
        S=FULL["S"], H=FULL["H"], NQH=FULL["NQH"],
    )
    return out
